# revision 2
# baseline (speedup 1.0000x reference)
"""GCN layer (segment-sum + global softmax + leaky-relu) on 8 TRN2 NeuronCores.

Full computation runs on-device as one SPMD Bass/Tile kernel:
  - nodes sharded 12500/core (hint: shard nodes, edges by dest row)
  - bf16 embedding rows gathered from HBM by indirect DMA (one 128-row
    gather instruction per edge block; only [128,1] offset APs work on HW)
  - per-128-row-window one-hot matrices (iota is_equal rowrel, times val)
    built on DVE, merged on the TensorEngine into PSUM, evicted by ACT
  - z = agg @ att_weight via DVE accumulate; global softmax denominator via
    ones-matmul reduction + cross-core AllReduce (max subtraction skipped:
    z is O(10), exp is safe in fp32, softmax is shift-invariant)
  - out = leaky_relu(agg * score); host only packs streams and reassembles.
"""
from contextlib import ExitStack

import numpy as np
import ml_dtypes

P = 128
D = 64
N_NODES = 100000
N_CORES = 8
LEAK = 0.2


# ---------------------------------------------------------------- legalizer
def _legalize_multiwait(nc, max_waits=1):
    """This container's walrus supports one semaphore wait per instruction;
    hoist extras onto standalone EventSemaphore instructions."""
    from concourse import mybir

    for f in nc.m.functions:
        for b in f.blocks:
            il = list(b.instructions)
            out = []
            changed = False
            for inst in il:
                si = inst.sync_info
                if si is not None and len(si.on_wait) > max_waits:
                    waits = list(si.on_wait)
                    head, keep = waits[:-max_waits], waits[-max_waits:]
                    for wt in head:
                        ev = mybir.InstEventSemaphore(
                            name=f"lgw-{nc.next_id()}", ins=[], outs=[],
                            sync_info=mybir.SyncInfo(on_wait=[wt], on_update=[]),
                        )
                        ev.engine = inst.engine
                        out.append(ev)
                    si.on_wait = keep
                    inst.sync_info = si
                    changed = True
                out.append(inst)
            if changed:
                b.instructions = out


# ------------------------------------------------------------ host schedule
def _make_schedule(rows, cols, vals):
    E = rows.shape[0]
    shard = N_NODES // N_CORES
    nwin = -(-shard // P)

    core = rows // shard
    local = rows % shard
    win = local // P
    rowrel = local % P

    key = (core * nwin + win).astype(np.int64)
    order = np.argsort(key, kind="stable")
    ks = key[order]
    cols_s = cols[order]
    vals_s = vals[order]
    rowrel_s = rowrel[order]

    cnt = np.bincount(key, minlength=N_CORES * nwin).reshape(N_CORES, nwin)
    B = np.maximum(1, -(-cnt.max(axis=0) // P)).astype(np.int64)
    blk_start = np.concatenate([[0], np.cumsum(B)])
    NB = int(blk_start[-1])

    first = np.r_[0, np.flatnonzero(np.diff(ks)) + 1]
    runlen = np.diff(np.r_[first, len(ks)])
    rank = np.arange(E) - np.repeat(first, runlen)
    win_s = ks % nwin
    core_s = ks // nwin
    slot = blk_start[win_s] * P + rank

    S = NB * P
    per_core = []
    for k in range(N_CORES):
        m = core_s == k
        sl = slot[m]
        idx = np.zeros(S, dtype=np.int32)
        rr = np.zeros(S, dtype=np.float32)
        vv = np.zeros(S, dtype=np.float32)
        idx[sl] = cols_s[m]
        rr[sl] = rowrel_s[m]
        vv[sl] = vals_s[m]
        per_core.append({
            "idx": np.ascontiguousarray(idx.reshape(NB, P).T),
            "rowrel": np.ascontiguousarray(rr.reshape(NB, P).T).astype(
                ml_dtypes.bfloat16),
            "val": np.ascontiguousarray(vv.reshape(NB, P).T).astype(
                ml_dtypes.bfloat16),
        })

    sched = {"shard": shard, "nwin": nwin, "NB": NB,
             "B": B.tolist(), "blk_start": blk_start.tolist()}
    return sched, per_core


# ------------------------------------------------------------ device kernel
def _build_nc(sched, CB=16, g_bufs=10, oh_bufs=3, ps_bufs=4):
    import concourse.bass as bass
    import concourse.tile as tile
    from concourse import mybir

    nwin = sched["nwin"]
    NB = sched["NB"]
    B = sched["B"]
    blk_start = sched["blk_start"]
    n_tbl = N_NODES + 4
    dt = mybir.dt

    nc = bass.Bass("TRN2", target_bir_lowering=False, debug=False,
                   num_devices=N_CORES)
    tbl_t = nc.dram_tensor("tbl", [n_tbl, D], dt.bfloat16, kind="ExternalInput")
    idx_t = nc.dram_tensor("idx", [P, NB], dt.int32, kind="ExternalInput")
    rr_t = nc.dram_tensor("rowrel", [P, NB], dt.bfloat16, kind="ExternalInput")
    vv_t = nc.dram_tensor("val", [P, NB], dt.bfloat16, kind="ExternalInput")
    iota_t = nc.dram_tensor("iota", [P, P], dt.bfloat16, kind="ExternalInput")
    w_t = nc.dram_tensor("w", [P, D], dt.float32, kind="ExternalInput")
    out_t = nc.dram_tensor("out", [nwin * P, D], dt.float32,
                           kind="ExternalOutput")

    with tile.TileContext(nc) as tc, ExitStack() as ctx:
        sb = ctx.enter_context(tc.tile_pool(name="sb", bufs=1))
        gpool = ctx.enter_context(tc.tile_pool(name="g", bufs=g_bufs))
        ohpool = ctx.enter_context(tc.tile_pool(name="oh", bufs=oh_bufs))
        t1pool = ctx.enter_context(tc.tile_pool(name="t1", bufs=2))
        pspool = ctx.enter_context(tc.tile_pool(name="ps", bufs=ps_bufs,
                                                space="PSUM"))
        ps2pool = ctx.enter_context(tc.tile_pool(name="ps2", bufs=2,
                                                 space="PSUM"))
        dram = ctx.enter_context(tc.tile_pool(name="dram", bufs=2, space="DRAM"))

        idx_sb = sb.tile([P, NB], dt.int32)
        nc.sync.dma_start(idx_sb[:], idx_t.ap())
        rr_sb = sb.tile([P, NB], dt.bfloat16)
        nc.sync.dma_start(rr_sb[:], rr_t.ap())
        vv_sb = sb.tile([P, NB], dt.bfloat16)
        nc.sync.dma_start(vv_sb[:], vv_t.ap())
        iota_sb = sb.tile([P, P], dt.bfloat16)
        nc.sync.dma_start(iota_sb[:], iota_t.ap())
        w_sb = sb.tile([P, D], dt.float32)
        nc.sync.dma_start(w_sb[:], w_t.ap())

        agg = sb.tile([P, nwin, D], dt.float32)
        zcol = sb.tile([P, nwin], dt.float32)

        nchunks = -(-NB // CB)
        chunk_tiles = [None] * nchunks

        def ensure_chunk(j):
            # only [P,1] offset APs work on real HW -> one DMA per block
            if chunk_tiles[j] is None:
                t = gpool.tile([P, CB, D], dt.bfloat16, tag="G")
                nb = min(CB, NB - j * CB)
                for c in range(nb):
                    nc.gpsimd.indirect_dma_start(
                        out=t[:, c, :], out_offset=None, in_=tbl_t.ap(),
                        in_offset=bass.IndirectOffsetOnAxis(
                            ap=idx_sb[:, j * CB + c:j * CB + c + 1], axis=0),
                    )
                chunk_tiles[j] = t
            return chunk_tiles[j]

        for w in range(nwin):
            b0, nb_w = blk_start[w], B[w]
            oh = ohpool.tile([P, nb_w, P], dt.bfloat16, tag="oh")
            iota_b = iota_sb[:].unsqueeze(1).broadcast_to([P, nb_w, P])
            rr_b = rr_sb[:, b0:b0 + nb_w].unsqueeze(2).broadcast_to([P, nb_w, P])
            vv_b = vv_sb[:, b0:b0 + nb_w].unsqueeze(2).broadcast_to([P, nb_w, P])
            nc.vector.tensor_tensor(out=oh[:], in0=iota_b, in1=rr_b,
                                    op=mybir.AluOpType.is_equal)
            nc.vector.tensor_tensor(out=oh[:], in0=oh[:], in1=vv_b,
                                    op=mybir.AluOpType.mult)

            ps = pspool.tile([P, D], dt.float32, tag="ps")
            for i in range(nb_w):
                b = b0 + i
                g = ensure_chunk(b // CB)
                nc.tensor.matmul(
                    out=ps[:], lhsT=oh[:, i, :], rhs=g[:, b % CB, :],
                    start=(i == 0), stop=(i == nb_w - 1),
                )
            nc.scalar.activation(out=agg[:, w, :], in_=ps[:],
                                 func=mybir.ActivationFunctionType.Copy)
            t1 = t1pool.tile([P, D], dt.float32, tag="t1")
            nc.vector.scalar_tensor_tensor(
                out=t1[:], in0=agg[:, w, :], scalar=1.0, in1=w_sb[:],
                op0=mybir.AluOpType.mult, op1=mybir.AluOpType.mult,
                accum_out=zcol[:, w:w + 1],
            )

        ex = sb.tile([P, nwin], dt.float32)
        sum_p = sb.tile([P, 1], dt.float32)
        nc.scalar.activation(out=ex[:], in_=zcol[:],
                             func=mybir.ActivationFunctionType.Exp,
                             accum_out=sum_p[:])
        ones_c = sb.tile([P, 1], dt.float32)
        nc.vector.memset(ones_c[:], 1.0)
        ones_r = sb.tile([1, P], dt.float32)
        nc.vector.memset(ones_r[:], 1.0)
        tot_ps = ps2pool.tile([1, 1], dt.float32, tag="tot")
        nc.tensor.matmul(out=tot_ps[:], lhsT=ones_c[:], rhs=sum_p[:],
                         start=True, stop=True)
        tot_sb = sb.tile([1, 1], dt.float32)
        nc.vector.tensor_copy(out=tot_sb[:], in_=tot_ps[:])

        cin = dram.tile([1, 1], dt.float32)
        cout = dram.tile([1, 1], dt.float32)
        nc.sync.dma_start(cin[:], tot_sb[:])
        nc.gpsimd.collective_compute(
            "AllReduce", mybir.AluOpType.add,
            replica_groups=[list(range(N_CORES))],
            ins=[cin.opt()], outs=[cout.opt()],
        )
        den_sb = sb.tile([1, 1], dt.float32)
        nc.sync.dma_start(den_sb[:], cout[:])

        den_ps = ps2pool.tile([P, 1], dt.float32, tag="den")
        nc.tensor.matmul(out=den_ps[:], lhsT=ones_r[:], rhs=den_sb[:],
                         start=True, stop=True)
        den_all = sb.tile([P, 1], dt.float32)
        nc.vector.tensor_copy(out=den_all[:], in_=den_ps[:])
        inv = sb.tile([P, 1], dt.float32)
        nc.vector.reciprocal(inv[:], den_all[:])
        sc = sb.tile([P, nwin], dt.float32)
        nc.vector.tensor_scalar_mul(sc[:], ex[:], inv[:, 0:1])

        for w in range(nwin):
            t1 = t1pool.tile([P, D], dt.float32, tag="t1")
            nc.vector.tensor_scalar_mul(t1[:], agg[:, w, :], sc[:, w:w + 1])
            nc.vector.scalar_tensor_tensor(
                out=agg[:, w, :], in0=t1[:], scalar=LEAK, in1=t1[:],
                op0=mybir.AluOpType.mult, op1=mybir.AluOpType.max,
            )
        out_v = out_t.ap().rearrange("(w p) d -> p w d", p=P)
        nc.sync.dma_start(out_v, agg[:])

    _legalize_multiwait(nc)
    return nc


_last_exec_ns = None


def kernel(adj_rows, adj_cols, adj_vals, embeds, att_weight):
    global _last_exec_ns
    import os

    rows = np.asarray(adj_rows).astype(np.int64)
    cols = np.asarray(adj_cols).astype(np.int64)
    vals = np.asarray(adj_vals, dtype=np.float32)
    embeds = np.asarray(embeds, dtype=np.float32)
    att_w = np.asarray(att_weight, dtype=np.float32)

    sched, per_core = _make_schedule(rows, cols, vals)

    tbl = np.zeros((N_NODES + 4, D), dtype=ml_dtypes.bfloat16)
    tbl[:N_NODES] = embeds.astype(ml_dtypes.bfloat16)
    iota = np.ascontiguousarray(
        np.broadcast_to(np.arange(P, dtype=np.float32), (P, P))
    ).astype(ml_dtypes.bfloat16)
    w_rep = np.ascontiguousarray(
        np.broadcast_to(att_w.reshape(-1), (P, D))
    ).astype(np.float32)
    shared = {"tbl": tbl, "iota": iota, "w": w_rep}

    nc = _build_nc(sched)
    in_maps = [dict(shared, **pc) for pc in per_core]

    trace = os.environ.get("GCN_TRACE", "0") == "1"
    if trace:
        try:
            import ntff_hook
            ntff_hook.install()
        except Exception:
            trace = False

    from concourse.bass_utils import run_bass_kernel_spmd
    res = run_bass_kernel_spmd(nc, in_maps, list(range(N_CORES)), trace=trace)
    _last_exec_ns = res.exec_time_ns

    shard = sched["shard"]
    out = np.concatenate(
        [res.results[k]["out"][:shard] for k in range(N_CORES)], axis=0)
    return np.ascontiguousarray(out, dtype=np.float32)
